# revision 27
# baseline (speedup 1.0000x reference)
"""Trainium2 Bass kernel for the attention-like exp/reduce problem.

Math (per batch element b, fully data-parallel across 8 cores):
    M[p, q]      = exp(dec[p] * enc[t, q])            (256x256 per timestep t)
    colsum[t,q]  = sum_p M[p, q]
    rowsum[t,q]  = sum_r exp(dec[q] * enc[t, r])
    out[q]       = sum_t enc[t,q] * colsum[t,q] / rowsum[t,q]

Implementation notes:
  * M is materialized once per core in orientation [i (dec idx, partition), (t, j) free]
    as exp(dec[i] * enc[t, j]): enc rows are broadcast across all 128 partitions by a
    0-stride DMA, then one ACT instruction per (chunk, i-half) applies
    exp(scale * x) with per-partition scale = dec[i].
  * M is stored in bf16 (band entries 0/1 are exact; M's rounding noise averages
    out in the 256-term sums; both colsum and rowsum share the same rounded M,
    keeping the ratio well-conditioned).
  * rowsum[t, q] == per-partition segmented free-axis sums of M: a bf16 pairwise
    fold (tensor_tensor add, fast packed mode) followed by a DVE tensor_reduce with
    a 3D access pattern (innermost axis) writes rowsum^T in [q, t] layout directly.
  * colsum[t, q] == partition-axis sums of M. Done on the tensor engine: stationary
    lhsT is a slice of a "band" matrix (all-ones column), lhsT[i, m] = 1 iff m == t,
    so out[m, :] += (m == t) ? colsum_t : 0 accumulates the whole [t, q] colsum
    matrix into a single PSUM tile across 256 matmuls.
  * combine: reciprocal on DVE, PE transpose of rowsum^T, scalar_tensor_tensor
    reading colsum straight from PSUM, final all-ones-column matmul contracts over
    t, DMA out. Chunk sizes ramp up/down ([4,4,8,8]...[12,4]) to fill and drain the
    ACT/DVE/PE/DMA pipeline quickly.
"""

import sys

sys.path.insert(0, "/opt/trn_rl_repo")

import numpy as np

import concourse.bass as bass
import concourse.bacc as bacc
import concourse.tile as tile
from concourse import mybir
from concourse.bass_utils import run_bass_kernel_spmd

B, T, D = 8, 128, 256
NCORES = 8
TC = 16  # timesteps per chunk
ACCUM_T = 0  # timesteps per (chunk, half) routed via ACT accum_out instead of DVE
F32 = mybir.dt.float32
F32R = mybir.dt.float32r
BF16 = mybir.dt.bfloat16
EXP = mybir.ActivationFunctionType.Exp


def _band_np():
    import ml_dtypes
    band = np.zeros((128, 257), dtype=ml_dtypes.bfloat16)
    band[:, 128] = 1.0
    return band


def _ident_np():
    return np.eye(128, dtype=np.float32)


def build_nc():
    nc = bacc.Bacc("TRN2")
    dec2 = nc.dram_tensor("dec2", [128, 2], F32, kind="ExternalInput").ap()
    enc = nc.dram_tensor("enc", [T, D], F32, kind="ExternalInput").ap()
    band = nc.dram_tensor("band", [128, 257], BF16, kind="ExternalInput").ap()
    onescol = nc.dram_tensor("onescol", [128, 1], BF16, kind="ExternalInput").ap()
    ident = nc.dram_tensor("ident", [128, 128], F32, kind="ExternalInput").ap()
    out = nc.dram_tensor("out", [1, D], F32, kind="ExternalOutput").ap()

    ramp, tail = [4, 4, 8, 8], [12, 4]
    mid = T - sum(ramp) - sum(tail)
    chunk_sizes = ramp + [TC] * (mid // TC) + ([mid % TC] if mid % TC else []) + tail
    assert sum(chunk_sizes) == T, chunk_sizes
    n_cs_mms = 2 * T  # two i-halves per timestep

    with tile.TileContext(nc) as tc:
        with (
            tc.tile_pool(name="const", bufs=1) as constp,
            tc.tile_pool(name="bc", bufs=4) as bcp,
            tc.tile_pool(name="m", bufs=4) as mp,
            tc.tile_pool(name="fold", bufs=3) as foldp,
            tc.tile_pool(name="cs", bufs=1, space="PSUM") as csp,
            tc.tile_pool(name="tp", bufs=2, space="PSUM") as tpp,
        ):
            warm = constp.tile([128, 1], F32, tag="warm")
            nc.vector.memset(warm[:], 0.0)
            nc.scalar.activation(warm[:], warm[:], EXP)
            dec_sb = constp.tile([128, 2], F32, tag="dec")
            nc.gpsimd.dma_start(dec_sb[:], dec2)
            enc_sb = constp.tile([T, D], F32, tag="enc")
            nc.gpsimd.dma_start(enc_sb[:], enc)
            band_sb = constp.tile([128, 257], BF16, tag="band")
            nc.gpsimd.dma_start(band_sb[:], band)
            onescol_sb = constp.tile([128, 1], BF16, tag="onescol")
            nc.gpsimd.dma_start(onescol_sb[:], onescol)
            ident_sb = constp.tile([128, 128], F32, tag="ident")
            nc.gpsimd.dma_start(ident_sb[:], ident)

            # rowsum^T accumulators: [q (partition), t] for q in [0,128) / [128,256)
            rsT = [
                constp.tile([128, T], F32, tag="rsT_lo", name="rsT_lo"),
                constp.tile([128, T], F32, tag="rsT_hi", name="rsT_hi"),
            ]
            cs_ps = csp.tile([128, D], F32, tag="cs")  # colsum [t, q], PSUM accum
            rr_early = [
                constp.tile([128, T], F32, tag="rr_lo", name="rr_lo_e"),
                constp.tile([128, T], F32, tag="rr_hi", name="rr_hi_e"),
            ]

            mm_idx = 0
            t0 = 0
            for c, tcnt in enumerate(chunk_sizes):
                W = tcnt * D
                bc = bcp.tile([128, TC * D], F32, tag="bc")
                src = enc[t0 : t0 + tcnt, :].rearrange("t d -> (t d)")
                nc.sync.dma_start(bc[:, :W], src.partition_broadcast(128))

                for half in range(2):
                    m_t = mp.tile([128, TC * D], BF16, tag=f"m{half}")
                    scale_ap = dec_sb[:, half : half + 1]
                    a = min(ACCUM_T, tcnt)
                    for tt in range(a):
                        nc.scalar.activation(
                            m_t[:, tt * D : (tt + 1) * D],
                            bc[:, tt * D : (tt + 1) * D],
                            EXP,
                            scale=scale_ap,
                            accum_out=rsT[half][:, t0 + tt : t0 + tt + 1],
                        )
                    if tcnt > a:
                        nc.scalar.activation(
                            m_t[:, a * D : W],
                            bc[:, a * D : W],
                            EXP,
                            scale=scale_ap,
                        )
                        m_v = m_t[:, a * D : W].rearrange(
                            "p (t d) -> p t d", d=D
                        )
                        ft = foldp.tile(
                            [128, TC * (D // 2)], BF16, tag=f"f{half}",
                            name=f"f{half}",
                        )
                        f_v = ft[:, : (tcnt - a) * (D // 2)].rearrange(
                            "p (t d) -> p t d", d=D // 2
                        )
                        nc.vector.tensor_tensor(
                            f_v,
                            m_v[:, :, 0 : D // 2],
                            m_v[:, :, D // 2 : D],
                            op=mybir.AluOpType.add,
                        )
                        nc.vector.tensor_reduce(
                            rsT[half][:, t0 + a : t0 + tcnt],
                            f_v,
                            axis=mybir.AxisListType.X,
                            op=mybir.AluOpType.add,
                        )
                    for tt in range(tcnt):
                        t_abs = t0 + tt
                        nc.tensor.matmul(
                            cs_ps[:],
                            band_sb[:, 128 - t_abs : 256 - t_abs],
                            m_t[:, tt * D : (tt + 1) * D],
                            start=(mm_idx == 0),
                            stop=(mm_idx == n_cs_mms - 1),
                        )
                        mm_idx += 1
                t0 += tcnt

            # ---- epilogue ----
            rr = rr_early
            nc.vector.reciprocal(rr[0][:], rsT[0][:])
            nc.vector.reciprocal(rr[1][:], rsT[1][:])
            tmp = constp.tile([T, D], F32, tag="tmp")
            rrT = constp.tile([T, D], F32, tag="rrT")
            for half in range(2):
                tp = tpp.tile([128, 128], F32, tag="tp")
                nc.tensor.transpose(tp[:], rr[half][:], ident_sb[:])
                sl = slice(half * 128, (half + 1) * 128)
                nc.vector.tensor_copy(rrT[:, sl], tp[:])
                # tmp = (cs_ps * 1.0) * rrT  -- reads cs from PSUM directly
                nc.vector.scalar_tensor_tensor(
                    tmp[:, sl],
                    cs_ps[:, sl],
                    1.0,
                    rrT[:, sl],
                    op0=mybir.AluOpType.mult,
                    op1=mybir.AluOpType.mult,
                )
            contrib = constp.tile([T, D], BF16, tag="contrib")
            nc.vector.tensor_mul(contrib[:], tmp[:], enc_sb[:])
            fin = tpp.tile([1, D], F32, tag="fin")
            nc.tensor.matmul(
                fin[:], onescol_sb[:], contrib[:], start=True, stop=True
            )
            out_sb = constp.tile([1, D], F32, tag="out_sb")
            nc.scalar.copy(out_sb[:], fin[:])
            nc.sync.dma_start(out, out_sb[:])
    nc.compile()
    return nc


_NC_CACHE = None


def _get_nc():
    global _NC_CACHE
    if _NC_CACHE is None:
        _NC_CACHE = build_nc()
    return _NC_CACHE


def make_in_maps(dec_t: np.ndarray, enc_out: np.ndarray):
    band = _band_np()
    ident = _ident_np()
    in_maps = []
    for b in range(B):
        dec2 = np.stack(
            [dec_t[b, :128], dec_t[b, 128:]], axis=1
        ).astype(np.float32)  # [128, 2]
        in_maps.append(
            {
                "dec2": np.ascontiguousarray(dec2),
                "enc": np.ascontiguousarray(enc_out[b]).astype(np.float32),
                "band": band,
                "onescol": np.ones((128, 1), dtype=_band_np().dtype),
                "ident": ident,
            }
        )
    return in_maps


def run(dec_t: np.ndarray, enc_out: np.ndarray, **kwargs):
    """Run on all 8 cores; returns ([B, D] output, BassKernelResults)."""
    nc = _get_nc()
    res = run_bass_kernel_spmd(
        nc, make_in_maps(dec_t, enc_out), core_ids=list(range(NCORES)), **kwargs
    )
    out = np.stack([np.asarray(r["out"]).reshape(D) for r in res.results], axis=0)
    return out.astype(np.float32), res


def kernel(dec_t: np.ndarray, enc_out: np.ndarray) -> np.ndarray:
    dec_t = np.asarray(dec_t, dtype=np.float32)
    enc_out = np.asarray(enc_out, dtype=np.float32)
    out, _ = run(dec_t, enc_out)
    return out


# revision 28
# speedup vs baseline: 1.0067x; 1.0067x over previous
"""Trainium2 Bass kernel for the attention-like exp/reduce problem.

Math (per batch element b, fully data-parallel across 8 cores):
    M[p, q]      = exp(dec[p] * enc[t, q])            (256x256 per timestep t)
    colsum[t,q]  = sum_p M[p, q]
    rowsum[t,q]  = sum_r exp(dec[q] * enc[t, r])
    out[q]       = sum_t enc[t,q] * colsum[t,q] / rowsum[t,q]

Implementation notes:
  * M is materialized once per core in orientation [i (dec idx, partition), (t, j) free]
    as exp(dec[i] * enc[t, j]): enc rows are broadcast across all 128 partitions by a
    0-stride DMA, then one ACT instruction per (chunk, i-half) applies
    exp(scale * x) with per-partition scale = dec[i].
  * M is stored in bf16 (band entries 0/1 are exact; M's rounding noise averages
    out in the 256-term sums; both colsum and rowsum share the same rounded M,
    keeping the ratio well-conditioned).
  * rowsum[t, q] == per-partition segmented free-axis sums of M: a bf16 pairwise
    fold (tensor_tensor add, fast packed mode) followed by a DVE tensor_reduce with
    a 3D access pattern (innermost axis) writes rowsum^T in [q, t] layout directly.
  * colsum[t, q] == partition-axis sums of M. Done on the tensor engine: stationary
    lhsT is a slice of a "band" matrix (all-ones column), lhsT[i, m] = 1 iff m == t,
    so out[m, :] += (m == t) ? colsum_t : 0 accumulates the whole [t, q] colsum
    matrix into a single PSUM tile across 256 matmuls.
  * combine: reciprocal on DVE, PE transpose of rowsum^T, scalar_tensor_tensor
    reading colsum straight from PSUM, final all-ones-column matmul contracts over
    t, DMA out. Chunk sizes ramp up/down ([4,4,8,8]...[12,4]) to fill and drain the
    ACT/DVE/PE/DMA pipeline quickly.
"""

import sys

sys.path.insert(0, "/opt/trn_rl_repo")

import numpy as np

import concourse.bass as bass
import concourse.bacc as bacc
import concourse.tile as tile
from concourse import mybir
from concourse.bass_utils import run_bass_kernel_spmd

# The agent image's antenv package lacks axon_hooks; if BASS_TRACE is set in the
# environment, run_bass_kernel_spmd would die on the import. Provide a stub that
# reports "no hook" so tracing degrades gracefully instead. (A real hook installed
# earlier, e.g. by a profiling harness, is left untouched.)
try:
    import antenv.axon_hooks  # noqa: F401
except ImportError:
    import types

    import antenv

    _hooks = types.ModuleType("antenv.axon_hooks")
    _hooks.get_axon_ntff_profile_hook = lambda: None
    _hooks.set_axon_ntff_profile_hook = lambda h: None
    sys.modules["antenv.axon_hooks"] = _hooks
    antenv.axon_hooks = _hooks

B, T, D = 8, 128, 256
NCORES = 8
TC = 16  # timesteps per chunk
ACCUM_T = 0  # timesteps per (chunk, half) routed via ACT accum_out instead of DVE
F32 = mybir.dt.float32
F32R = mybir.dt.float32r
BF16 = mybir.dt.bfloat16
EXP = mybir.ActivationFunctionType.Exp


def _band_np():
    import ml_dtypes
    band = np.zeros((128, 257), dtype=ml_dtypes.bfloat16)
    band[:, 128] = 1.0
    return band


def _ident_np():
    return np.eye(128, dtype=np.float32)


def build_nc():
    nc = bacc.Bacc("TRN2")
    dec2 = nc.dram_tensor("dec2", [128, 2], F32, kind="ExternalInput").ap()
    enc = nc.dram_tensor("enc", [T, D], F32, kind="ExternalInput").ap()
    band = nc.dram_tensor("band", [128, 257], BF16, kind="ExternalInput").ap()
    onescol = nc.dram_tensor("onescol", [128, 1], BF16, kind="ExternalInput").ap()
    ident = nc.dram_tensor("ident", [128, 128], F32, kind="ExternalInput").ap()
    out = nc.dram_tensor("out", [1, D], F32, kind="ExternalOutput").ap()

    ramp, tail = [4, 4, 8, 8], [12, 4]
    mid = T - sum(ramp) - sum(tail)
    chunk_sizes = ramp + [TC] * (mid // TC) + ([mid % TC] if mid % TC else []) + tail
    assert sum(chunk_sizes) == T, chunk_sizes
    n_cs_mms = 2 * T  # two i-halves per timestep

    with tile.TileContext(nc) as tc:
        with (
            tc.tile_pool(name="const", bufs=1) as constp,
            tc.tile_pool(name="bc", bufs=4) as bcp,
            tc.tile_pool(name="m", bufs=4) as mp,
            tc.tile_pool(name="fold", bufs=3) as foldp,
            tc.tile_pool(name="cs", bufs=1, space="PSUM") as csp,
            tc.tile_pool(name="tp", bufs=2, space="PSUM") as tpp,
        ):
            warm = constp.tile([128, 1], F32, tag="warm")
            nc.vector.memset(warm[:], 0.0)
            nc.scalar.activation(warm[:], warm[:], EXP)
            dec_sb = constp.tile([128, 2], F32, tag="dec")
            nc.gpsimd.dma_start(dec_sb[:], dec2)
            enc_sb = constp.tile([T, D], F32, tag="enc")
            nc.gpsimd.dma_start(enc_sb[:], enc)
            band_sb = constp.tile([128, 257], BF16, tag="band")
            nc.gpsimd.dma_start(band_sb[:], band)
            onescol_sb = constp.tile([128, 1], BF16, tag="onescol")
            nc.gpsimd.dma_start(onescol_sb[:], onescol)
            ident_sb = constp.tile([128, 128], F32, tag="ident")
            nc.gpsimd.dma_start(ident_sb[:], ident)

            # rowsum^T accumulators: [q (partition), t] for q in [0,128) / [128,256)
            rsT = [
                constp.tile([128, T], F32, tag="rsT_lo", name="rsT_lo"),
                constp.tile([128, T], F32, tag="rsT_hi", name="rsT_hi"),
            ]
            cs_ps = csp.tile([128, D], F32, tag="cs")  # colsum [t, q], PSUM accum
            rr_early = [
                constp.tile([128, T], F32, tag="rr_lo", name="rr_lo_e"),
                constp.tile([128, T], F32, tag="rr_hi", name="rr_hi_e"),
            ]

            mm_idx = 0
            t0 = 0
            for c, tcnt in enumerate(chunk_sizes):
                W = tcnt * D
                bc = bcp.tile([128, TC * D], F32, tag="bc")
                src = enc[t0 : t0 + tcnt, :].rearrange("t d -> (t d)")
                nc.sync.dma_start(bc[:, :W], src.partition_broadcast(128))

                for half in range(2):
                    m_t = mp.tile([128, TC * D], BF16, tag=f"m{half}")
                    scale_ap = dec_sb[:, half : half + 1]
                    a = min(ACCUM_T, tcnt)
                    for tt in range(a):
                        nc.scalar.activation(
                            m_t[:, tt * D : (tt + 1) * D],
                            bc[:, tt * D : (tt + 1) * D],
                            EXP,
                            scale=scale_ap,
                            accum_out=rsT[half][:, t0 + tt : t0 + tt + 1],
                        )
                    if tcnt > a:
                        nc.scalar.activation(
                            m_t[:, a * D : W],
                            bc[:, a * D : W],
                            EXP,
                            scale=scale_ap,
                        )
                        m_v = m_t[:, a * D : W].rearrange(
                            "p (t d) -> p t d", d=D
                        )
                        ft = foldp.tile(
                            [128, TC * (D // 2)], BF16, tag=f"f{half}",
                            name=f"f{half}",
                        )
                        f_v = ft[:, : (tcnt - a) * (D // 2)].rearrange(
                            "p (t d) -> p t d", d=D // 2
                        )
                        nc.vector.tensor_tensor(
                            f_v,
                            m_v[:, :, 0 : D // 2],
                            m_v[:, :, D // 2 : D],
                            op=mybir.AluOpType.add,
                        )
                        nc.vector.tensor_reduce(
                            rsT[half][:, t0 + a : t0 + tcnt],
                            f_v,
                            axis=mybir.AxisListType.X,
                            op=mybir.AluOpType.add,
                        )
                    for tt in range(tcnt):
                        t_abs = t0 + tt
                        nc.tensor.matmul(
                            cs_ps[:],
                            band_sb[:, 128 - t_abs : 256 - t_abs],
                            m_t[:, tt * D : (tt + 1) * D],
                            start=(mm_idx == 0),
                            stop=(mm_idx == n_cs_mms - 1),
                        )
                        mm_idx += 1
                t0 += tcnt

            # ---- epilogue ----
            rr = rr_early
            nc.vector.reciprocal(rr[0][:], rsT[0][:])
            nc.vector.reciprocal(rr[1][:], rsT[1][:])
            tmp = constp.tile([T, D], F32, tag="tmp")
            rrT = constp.tile([T, D], F32, tag="rrT")
            for half in range(2):
                tp = tpp.tile([128, 128], F32, tag="tp")
                nc.tensor.transpose(tp[:], rr[half][:], ident_sb[:])
                sl = slice(half * 128, (half + 1) * 128)
                nc.vector.tensor_copy(rrT[:, sl], tp[:])
                # tmp = (cs_ps * 1.0) * rrT  -- reads cs from PSUM directly
                nc.vector.scalar_tensor_tensor(
                    tmp[:, sl],
                    cs_ps[:, sl],
                    1.0,
                    rrT[:, sl],
                    op0=mybir.AluOpType.mult,
                    op1=mybir.AluOpType.mult,
                )
            contrib = constp.tile([T, D], BF16, tag="contrib")
            nc.vector.tensor_mul(contrib[:], tmp[:], enc_sb[:])
            fin = tpp.tile([1, D], F32, tag="fin")
            nc.tensor.matmul(
                fin[:], onescol_sb[:], contrib[:], start=True, stop=True
            )
            out_sb = constp.tile([1, D], F32, tag="out_sb")
            nc.scalar.copy(out_sb[:], fin[:])
            nc.sync.dma_start(out, out_sb[:])
    nc.compile()
    return nc


_NC_CACHE = None


def _get_nc():
    global _NC_CACHE
    if _NC_CACHE is None:
        _NC_CACHE = build_nc()
    return _NC_CACHE


def make_in_maps(dec_t: np.ndarray, enc_out: np.ndarray):
    band = _band_np()
    ident = _ident_np()
    in_maps = []
    for b in range(B):
        dec2 = np.stack(
            [dec_t[b, :128], dec_t[b, 128:]], axis=1
        ).astype(np.float32)  # [128, 2]
        in_maps.append(
            {
                "dec2": np.ascontiguousarray(dec2),
                "enc": np.ascontiguousarray(enc_out[b]).astype(np.float32),
                "band": band,
                "onescol": np.ones((128, 1), dtype=_band_np().dtype),
                "ident": ident,
            }
        )
    return in_maps


def run(dec_t: np.ndarray, enc_out: np.ndarray, **kwargs):
    """Run on all 8 cores; returns ([B, D] output, BassKernelResults)."""
    nc = _get_nc()
    res = run_bass_kernel_spmd(
        nc, make_in_maps(dec_t, enc_out), core_ids=list(range(NCORES)), **kwargs
    )
    out = np.stack([np.asarray(r["out"]).reshape(D) for r in res.results], axis=0)
    return out.astype(np.float32), res


def kernel(dec_t: np.ndarray, enc_out: np.ndarray) -> np.ndarray:
    dec_t = np.asarray(dec_t, dtype=np.float32)
    enc_out = np.asarray(enc_out, dtype=np.float32)
    out, _ = run(dec_t, enc_out)
    return out
